# revision 17
# baseline (speedup 1.0000x reference)
"""GNN (2x SAGEConv + linear) Bass kernel for trn2, 8 NeuronCores.

Sharding: nodes partitioned across 8 cores (12500 each, dst-range).
Each layer: per-core windowed padded-CSR gathers of h[src] (dma_gather,
int16 windows of 25088 hcat rows), on-chip segment reduce (DVE strided),
batched unique-row dma_scatter_add into per-window DRAM accumulators,
dense combine + PE MLP.  One AllGather of h1 slices between layers.

v3: SWDGE queue rotation (4 Q7 core pairs in parallel for gather/scatter
descriptor generation), contiguous accumulator zeroing, 1/cnt folded into
phase-1 scatter, chunk-batched phase 2 (fat DVE/PE ops), feature-major
host-prepped h0T, bf16 h1T.
"""
import numpy as np

N = 100000
E = 1250000
HID = 64
P = 8
NPC = 12500          # nodes per core
RPC = 12544          # rows per core block (98 * 128), rows 12500+ are zero pads
NB = RPC // 128      # 98 blocks
WIN = 2 * RPC        # 25088 rows per gather window (2 rank blocks)
NW = 4               # windows
ZLOC = 12500         # local row inside a window that is guaranteed zero
ACCR = RPC + 128     # accumulator rows (tail rows are scratch)
MAXPOS = 4096        # max gather positions per call
MAXT = 16            # max tiles per gather call
CH = 8               # phase-2 tiles per chunk
QR = [3200, 3200, 3200, 2944]   # hc1 quarter rows (128-aligned tiles)
QS = [0, 3200, 6400, 9600]      # quarter start rows


def _wrap128(vals):
    """flat int16 stream -> [128, len/16] wrapped+replicated layout."""
    n = vals.shape[0]
    w16 = np.ascontiguousarray(vals.reshape(n // 16, 16).T)
    return np.tile(w16, (8, 1))


def _build_layer_meta(w_of, loc, dst, zloc_w):
    """Per-layer gather/scatter metadata (see v2 docstring)."""
    core = dst // NPC
    dstl = dst % NPC

    deg = np.zeros((P, NW, RPC), np.int32)
    np.add.at(deg, (core, w_of, dstl), 1)
    order = np.argsort(-deg, axis=2, kind="stable")  # [P, NW, RPC]
    deg_sorted = -np.sort(-deg, axis=2)
    tile_max = deg_sorted.reshape(P, NW, NB, 128).max(axis=3)
    D = tile_max.max(axis=0)                         # [NW, NB] shared

    groups = []
    for w in range(NW):
        gw = []
        cur, curpos = [], 0
        for t in range(NB):
            d = int(D[w, t])
            if d == 0:
                continue
            if cur and (curpos + d * 128 > MAXPOS or len(cur) >= MAXT):
                gw.append(cur)
                cur, curpos = [], 0
            cur.append(t)
            curpos += d * 128
        if cur:
            gw.append(cur)
        groups.append(gw)

    eorder = np.lexsort((loc, dstl, w_of, core))
    sc, sw, sd, sl = core[eorder], w_of[eorder], dstl[eorder], loc[eorder]
    key = ((sc * NW + sw) * RPC + sd).astype(np.int64)
    starts = np.searchsorted(key, np.arange(P * NW * RPC, dtype=np.int64))
    starts = np.append(starts, len(key))

    gidx_cores, sidx_cores = [], []
    for k in range(P):
        gparts, sparts = [], []
        for w in range(NW):
            od = order[k, w]
            for gt in groups[w]:
                for t in gt:
                    d = int(D[w, t])
                    nodes = od[t * 128:(t + 1) * 128]
                    blockg = np.full((d, 128), zloc_w[w], np.int32)
                    for p in range(128):
                        nloc = int(nodes[p])
                        s0 = starts[(k * NW + w) * RPC + nloc]
                        s1 = starts[(k * NW + w) * RPC + nloc + 1]
                        cnt = s1 - s0
                        if cnt:
                            blockg[:cnt, p] = sl[s0:s1]
                    gparts.append(blockg.reshape(-1))
                srows = np.concatenate(
                    [od[t * 128:(t + 1) * 128] for t in gt]).astype(np.int32)
                sparts.append(srows)
        gidx_cores.append(_wrap128(np.concatenate(gparts).astype(np.int16)))
        sidx_cores.append(_wrap128(np.concatenate(sparts).astype(np.int16)))
    return groups, D, gidx_cores, sidx_cores, order


def kernel(x, edge_index, edge_weight, emb, Wl1, bl1, Wr1, a1,
           Wl2, bl2, Wr2, a2, Wout, bout):
    import concourse.bacc as bacc
    import concourse.mybir as mybir
    import concourse.tile as tile
    from concourse.bass_utils import run_bass_kernel_spmd
    from concourse.masks import make_identity

    x = np.asarray(x).astype(np.int64)
    ei = np.asarray(edge_index).astype(np.int64)
    emb = np.asarray(emb, np.float32)
    Wl1 = np.asarray(Wl1, np.float32); Wr1 = np.asarray(Wr1, np.float32)
    Wl2 = np.asarray(Wl2, np.float32); Wr2 = np.asarray(Wr2, np.float32)
    Wout = np.asarray(Wout, np.float32)
    bl1 = np.asarray(bl1, np.float32); bl2 = np.asarray(bl2, np.float32)
    bout = np.asarray(bout, np.float32)
    a1f = float(np.asarray(a1)); a2f = float(np.asarray(a2))
    src, dst = ei[0], ei[1]

    # ---- host prep ------------------------------------------------------
    emb_hc = np.zeros((P * RPC, HID), np.float32)
    for r in range(P):
        emb_hc[r * RPC:r * RPC + NPC] = emb[r * NPC:(r + 1) * NPC]

    # per-core own h0 transposed (feature-major) [HID, RPC]
    h0T_own = np.zeros((P, HID, RPC), np.float32)
    for k in range(P):
        h0T_own[k, :, :NPC] = emb[x[k * NPC:(k + 1) * NPC]].T

    sid1 = x[src]
    w1 = sid1 // (2 * NPC)
    loc1 = RPC * ((sid1 // NPC) % 2) + sid1 % NPC
    g1, D1, gidx1, sidx1, ord1 = _build_layer_meta(w1, loc1, dst, [ZLOC] * NW)
    lr = src % NPC
    w2 = np.minimum(lr // 3200, 3)
    qr_a, qs_a = np.array(QR), np.array(QS)
    loc2 = (src // NPC) * qr_a[w2] + (lr - qs_a[w2])
    g2, D2, gidx2, sidx2, ord2 = _build_layer_meta(
        w2, loc2, dst, [P * QR[w] for w in range(NW)])

    # inverse counts (node order, [128, NB] partition-major)
    cnt = np.bincount(dst, minlength=N).astype(np.float32)
    invc = np.zeros((P, 128, NB), np.float32)
    for k in range(P):
        c = np.zeros(RPC, np.float32)
        c[:NPC] = 1.0 / np.maximum(cnt[k * NPC:(k + 1) * NPC], 1.0)
        invc[k] = c.reshape(NB, 128).T

    # ---- device program -------------------------------------------------
    f32, bf16, i16 = mybir.dt.float32, mybir.dt.bfloat16, mybir.dt.int16
    nc = bacc.Bacc(dynamic_dma_scratch_size=65536, num_swdge_queues=4)
    dp = nc.declare_dram_parameter
    embw = dp("embw", [P * RPC, HID], f32, isOutput=False)
    h0T_p = dp("h0T", [HID, RPC], f32, isOutput=False)
    gi1 = dp("gi1", list(gidx1[0].shape), i16, isOutput=False)
    si1 = dp("si1", list(sidx1[0].shape), i16, isOutput=False)
    gi2 = dp("gi2", list(gidx2[0].shape), i16, isOutput=False)
    si2 = dp("si2", list(sidx2[0].shape), i16, isOutput=False)
    invce_p = dp("invce", [128, NB, HID], f32, isOutput=False)
    wl1_p = dp("wl1", [HID, HID], f32, isOutput=False)
    wr1_p = dp("wr1", [HID, HID], f32, isOutput=False)
    wl2_p = dp("wl2", [HID, HID], f32, isOutput=False)
    wr2_p = dp("wr2", [HID, HID], f32, isOutput=False)
    wout_p = dp("wout", [HID, HID], f32, isOutput=False)
    bl1_p = dp("bl1t", [HID, 1], f32, isOutput=False)
    bl2_p = dp("bl2t", [HID, 1], f32, isOutput=False)
    bout_p = dp("boutr", [128, CH * HID], f32, isOutput=False)
    out_p = dp("out", [RPC, HID], f32, isOutput=True)

    acc_d = [[nc.dram_tensor(f"acc{li}_{w}", [ACCR, HID], f32)
              for w in range(NW)] for li in range(2)]
    hc1q = [nc.dram_tensor(f"hc1q{q}", [QR[q], HID], f32) for q in range(NW)]
    hcatq = [nc.dram_tensor(f"hcatq{q}", [P * QR[q] + 128, HID], f32,
                            addr_space="Shared") for q in range(NW)]

    AX = mybir.AxisListType.X
    ADD = mybir.AluOpType.add
    PRELU = mybir.ActivationFunctionType.Prelu

    qctr = [0]

    def next_q(ndesc):
        q = 1 + qctr[0] % 3
        qctr[0] += 1
        return q

    with tile.TileContext(nc) as tc:
        with tc.tile_pool(name="const", bufs=1) as cpool, \
             tc.tile_pool(name="big", bufs=1) as bpool, \
             tc.tile_pool(name="gio", bufs=3) as gpool, \
             tc.tile_pool(name="ph2", bufs=2) as qpool, \
             tc.tile_pool(name="ps", bufs=1, space="PSUM") as ppool:

            ident = cpool.tile([128, 128], f32)
            make_identity(nc, ident[:])
            ident_bf = cpool.tile([HID, HID], bf16)
            nc.vector.tensor_copy(ident_bf[:], ident[:HID, :HID])
            wl1_t = cpool.tile([HID, HID], f32); nc.sync.dma_start(wl1_t[:], wl1_p[:])
            wr1_t = cpool.tile([HID, HID], f32); nc.sync.dma_start(wr1_t[:], wr1_p[:])
            wl2_t = cpool.tile([HID, HID], f32); nc.sync.dma_start(wl2_t[:], wl2_p[:])
            wr2_t = cpool.tile([HID, HID], f32); nc.sync.dma_start(wr2_t[:], wr2_p[:])
            wr2b_t = cpool.tile([HID, HID], bf16)
            nc.vector.tensor_copy(wr2b_t[:], wr2_t[:])
            wout_t = cpool.tile([HID, HID], f32); nc.sync.dma_start(wout_t[:], wout_p[:])
            bl1_t = cpool.tile([HID, 1], f32); nc.sync.dma_start(bl1_t[:], bl1_p[:])
            bl2_t = cpool.tile([HID, 1], f32); nc.sync.dma_start(bl2_t[:], bl2_p[:])
            bout_t = cpool.tile([128, CH * HID], f32); nc.sync.dma_start(bout_t[:], bout_p[:])

            h1T = bpool.tile([HID, NB, 128], bf16)     # h1 transposed, own nodes
            zt = cpool.tile([128, HID], f32)
            nc.vector.memset(zt[:], 0.0)
            zbig = cpool.tile([128, 11 * HID], f32)
            nc.vector.memset(zbig[:], 0.0)

            def zero_accs(li):
                for w in range(NW):
                    # ACCR = 12672 = 128 * 99; partition p covers rows
                    # [99p, 99p+99) contiguously
                    dstv = acc_d[li][w][:].rearrange("(p b) f -> p (b f)", p=128)
                    for c in range(9):
                        nc.sync.dma_start(
                            dstv[:, c * 11 * HID:(c + 1) * 11 * HID], zbig[:])

            zero_accs(0)

            def phase1_setup(gi_p, si_p):
                gi_t = bpool.tile([128, gi_p.shape[1]], i16, tag="gi", name="gi_t")
                si_t = bpool.tile([128, si_p.shape[1]], i16, tag="si", name="si_t")
                nc.sync.dma_start(gi_t[:], gi_p[:])
                nc.sync.dma_start(si_t[:], si_p[:])
                return {"gi": gi_t, "si": si_t, "gcol": 0, "scol": 0}

            def phase1_window(st, li, groups, D, w, win):
                gi_t, si_t = st["gi"], st["si"]
                gcol, scol = st["gcol"], st["scol"]
                if True:
                    for gt in groups[w]:
                        npos = int(sum(D[w, t] for t in gt)) * 128
                        ncols = npos // 128
                        nt = len(gt)
                        g_t = gpool.tile([128, MAXPOS // 128, HID], f32, tag="g", name="g_t")
                        r_t = gpool.tile([128, MAXT, HID], f32, tag="r", name="r_t")
                        nc.gpsimd.dma_gather(
                            g_t[:, :ncols, :], win, gi_t[:, gcol:gcol + npos // 16],
                            npos, npos, HID, single_packet=False,
                            queue_num=next_q(npos))
                        off = 0
                        for i, t in enumerate(gt):
                            d = int(D[w, t])
                            view = g_t[:, off:off + d, :].rearrange("p d f -> p f d")
                            nc.vector.tensor_reduce(r_t[:, i, :], view, axis=AX, op=ADD)
                            off += d
                        nc.gpsimd.dma_scatter_add(
                            acc_d[li][w][:], r_t[:, :nt, :], si_t[:, scol:scol + nt * 8],
                            nt * 128, nt * 128, HID, single_packet=False,
                            queue_num=next_q(nt * 128))
                        gcol += npos // 16
                        scol += nt * 8
                st["gcol"], st["scol"] = gcol, scol

            def phase2(L, chunks=None):
                wl_t = wl1_t if L == 1 else wl2_t
                wr_t = wr1_t if L == 1 else wr2_t
                bl_t = bl1_t if L == 1 else bl2_t
                alpha = a1f if L == 1 else a2f
                if chunks is None:
                    chunks = [(c, min(CH, NB - c)) for c in range(0, NB, CH)]
                for c0, ct in chunks:
                    m4 = qpool.tile([128, NW, CH, HID], f32, tag="m4", name="m4")
                    for w in range(NW):
                        nc.scalar.dma_start(
                            m4[:, w, :ct, :],
                            acc_d[L - 1][w][c0 * 128:(c0 + ct) * 128]
                            .rearrange("(t p) f -> p t f", p=128))
                    invcc = qpool.tile([128, CH, HID], f32, tag="invcc", name="invcc")
                    nc.scalar.dma_start(invcc[:, :ct, :], invce_p[:, c0:c0 + ct, :])
                    mean0 = qpool.tile([128, CH, HID], f32, tag="mean0", name="mean0")
                    nc.vector.tensor_reduce(
                        mean0[:, :ct, :],
                        m4[:, :, :ct, :].rearrange("p w t f -> p t f w"),
                        axis=AX, op=ADD)
                    nc.vector.tensor_tensor(
                        mean0[:, :ct, :].rearrange("p t f -> p (t f)"),
                        mean0[:, :ct, :].rearrange("p t f -> p (t f)"),
                        invcc[:, :ct, :].rearrange("p t f -> p (t f)"),
                        op=mybir.AluOpType.mult)
                    # transpose ct tiles into psum (two banks of 4 tiles)
                    meanT = qpool.tile([HID, CH * 128], f32, tag="meanT", name="meanT")
                    for hb in range(0, ct, 4):
                        hn = min(4, ct - hb)
                        psT = ppool.tile([HID, 512], f32, tag=f"psT{(hb // 4) % 2}",
                                         name="psT")
                        for i in range(hn):
                            nc.tensor.transpose(
                                psT[:, i * 128:(i + 1) * 128],
                                mean0[:, hb + i, :], ident[:])
                        nc.vector.tensor_copy(
                            meanT[:, hb * 128:(hb + hn) * 128], psT[:, :hn * 128])
                    if L == 1:
                        hT = qpool.tile([HID, CH * 128], f32, tag="hT", name="hT")
                        nc.sync.dma_start(
                            hT[:, :ct * 128],
                            h0T_p[:, c0 * 128:(c0 + ct) * 128])
                    # matmuls in half-chunks of 4 tiles (psum 512 col limit)
                    for h in range(0, ct, 4):
                        hw = min(4, ct - h)
                        cols = slice(h * 128, (h + hw) * 128)
                        psC = ppool.tile([HID, 512], f32, tag=f"psC{(h // 4) % 2}",
                                         name="psC")
                        nc.tensor.matmul(psC[:, :hw * 128], wl_t[:], meanT[:, cols],
                                         start=True, stop=False)
                        if L == 1:
                            nc.tensor.matmul(psC[:, :hw * 128], wr_t[:], hT[:, cols],
                                             start=False, stop=True)
                        else:
                            nc.tensor.matmul(
                                psC[:, :hw * 128], wr2b_t[:],
                                h1T[:, c0 + h:c0 + h + hw, :]
                                .rearrange("f t n -> f (t n)"),
                                start=False, stop=True)
                        act_out = (h1T[:, c0 + h:c0 + h + hw, :] if L == 1 else
                                   h2T_t[:, h:h + hw, :])
                        nc.scalar.activation(
                            act_out.rearrange("f t n -> f (t n)"),
                            psC[:, :hw * 128], PRELU, bias=bl_t[:], alpha=alpha)
                    if L == 1:
                        # back-transpose to node-major and write hc1
                        psH = ppool.tile([128, CH * HID], bf16, tag="psH", name="psH")
                        for i in range(ct):
                            nc.tensor.transpose(
                                psH[:, i * HID:(i + 1) * HID],
                                h1T[:, c0 + i, :], ident_bf[:])
                        hc1c = qpool.tile([128, CH, HID], f32, tag="hc1c", name="hc1c")
                        nc.vector.tensor_copy(
                            hc1c[:, :ct, :].rearrange("p t f -> p (t f)"),
                            psH[:, :ct * HID])
                        # write into quarter tensors (split at boundaries)
                        r0, r1 = c0 * 128, (c0 + ct) * 128
                        for q in range(NW):
                            q0, q1 = QS[q], QS[q] + QR[q]
                            a, b = max(r0, q0), min(r1, q1)
                            if a >= b:
                                continue
                            t_a = (a - r0) // 128
                            t_b = (b - r0) // 128
                            nc.sync.dma_start(
                                hc1q[q][a - q0:b - q0]
                                .rearrange("(t p) f -> p t f", p=128),
                                hc1c[:, t_a:t_b, :])
                    else:
                        psE = ppool.tile([128, CH * HID], f32, tag="psE", name="psE")
                        for i in range(ct):
                            nc.tensor.matmul(
                                psE[:, i * HID:(i + 1) * HID],
                                h2T_t[:, i, :], wout_t[:], start=True, stop=True)
                        outc = qpool.tile([128, CH, HID], f32, tag="outc", name="outc")
                        nc.vector.tensor_tensor(
                            outc[:, :ct, :].rearrange("p t f -> p (t f)"),
                            psE[:, :ct * HID], bout_t[:, :ct * HID], op=ADD)
                        nc.sync.dma_start(
                            out_p[c0 * 128:(c0 + ct) * 128]
                            .rearrange("(t p) f -> p t f", p=128),
                            outc[:, :ct, :])

            h2T_t = bpool.tile([HID, CH, 128], f32)

            # zero the per-window pad blocks of the shared gather sources
            for q in range(NW):
                nc.sync.dma_start(hcatq[q][P * QR[q]:], zt[:])
            # ---- layer 1 ----
            st1 = phase1_setup(gi1, si1)
            for w in range(NW):
                phase1_window(st1, 0, g1, D1, w, embw[w * WIN:(w + 1) * WIN])
            zero_accs(1)
            st2 = phase1_setup(gi2, si2)
            # ---- phase2(L1) per quarter, interleaved with L2 windows ----
            QCH = [[(0, 8), (8, 8), (16, 8), (24, 1)],
                   [(25, 8), (33, 8), (41, 8), (49, 1)],
                   [(50, 8), (58, 8), (66, 8), (74, 1)],
                   [(75, 8), (83, 8), (91, 7)]]
            for q in range(NW):
                phase2(1, chunks=QCH[q])
                if q == 3:
                    # zero pad rows of h1 (nodes 12500..12543)
                    nc.sync.dma_start(hc1q[3][NPC - QS[3]:], zt[:RPC - NPC, :])
                nc.gpsimd.collective_compute(
                    "AllGather", mybir.AluOpType.bypass,
                    replica_groups=[list(range(P))],
                    ins=[hc1q[q][:]],
                    outs=[hcatq[q][:P * QR[q]]])
                phase1_window(st2, 1, g2, D2, q, hcatq[q][:])
            phase2(2)

    nc.compile()

    in_maps = []
    for k in range(P):
        in_maps.append({
            "embw": emb_hc, "h0T": h0T_own[k],
            "gi1": gidx1[k], "si1": sidx1[k],
            "gi2": gidx2[k], "si2": sidx2[k],
            "invce": np.repeat(invc[k].reshape(128, NB, 1), HID, axis=2),
            "wl1": Wl1, "wr1": Wr1, "wl2": Wl2, "wr2": Wr2, "wout": Wout,
            "bl1t": bl1.reshape(HID, 1), "bl2t": bl2.reshape(HID, 1),
            "boutr": np.tile(bout.reshape(1, HID), (128, CH)),
        })
    res = run_bass_kernel_spmd(nc, in_maps, list(range(P)))
    out = np.zeros((N, HID), np.float32)
    for k in range(P):
        out[k * NPC:(k + 1) * NPC] = res.results[k]["out"][:NPC]
    kernel.last_exec_time_ns = res.exec_time_ns
    return out


# revision 18
# speedup vs baseline: 1.2642x; 1.2642x over previous
"""GNN (2x SAGEConv + linear) Bass kernel for trn2, 8 NeuronCores.

Sharding: nodes partitioned across 8 cores (12500 each, dst-range).
Each layer: per-core windowed padded-CSR gathers of h[src] (dma_gather,
int16 windows of 25088 hcat rows), on-chip segment reduce (DVE strided),
batched unique-row dma_scatter_add into per-window DRAM accumulators,
dense combine + PE MLP.  One AllGather of h1 slices between layers.

v3: SWDGE queue rotation (4 Q7 core pairs in parallel for gather/scatter
descriptor generation), contiguous accumulator zeroing, 1/cnt folded into
phase-1 scatter, chunk-batched phase 2 (fat DVE/PE ops), feature-major
host-prepped h0T, bf16 h1T.
"""
import numpy as np

N = 100000
E = 1250000
HID = 64
P = 8
NPC = 12500          # nodes per core
RPC = 12544          # rows per core block (98 * 128), rows 12500+ are zero pads
NB = RPC // 128      # 98 blocks
WIN = 2 * RPC        # 25088 rows per gather window (2 rank blocks)
NW = 4               # windows
ZLOC = 12500         # local row inside a window that is guaranteed zero
ACCR = RPC + 128     # accumulator rows (tail rows are scratch)
MAXPOS = 4096        # max gather positions per call
MAXT = 16            # max tiles per gather call
CH = 8               # phase-2 tiles per chunk
QR = [3200, 3200, 3200, 2944]   # hc1 quarter rows (128-aligned tiles)
QS = [0, 3200, 6400, 9600]      # quarter start rows


def _wrap128(vals):
    """flat int16 stream -> [128, len/16] wrapped+replicated layout."""
    n = vals.shape[0]
    w16 = np.ascontiguousarray(vals.reshape(n // 16, 16).T)
    return np.tile(w16, (8, 1))


def _build_layer_meta(w_of, loc, dst, zloc_w):
    """Per-layer gather/scatter metadata (see v2 docstring)."""
    core = dst // NPC
    dstl = dst % NPC

    deg = np.zeros((P, NW, RPC), np.int32)
    np.add.at(deg, (core, w_of, dstl), 1)
    order = np.argsort(-deg, axis=2, kind="stable")  # [P, NW, RPC]
    deg_sorted = -np.sort(-deg, axis=2)
    tile_max = deg_sorted.reshape(P, NW, NB, 128).max(axis=3)
    D = tile_max.max(axis=0)                         # [NW, NB] shared

    groups = []
    for w in range(NW):
        gw = []
        cur, curpos = [], 0
        for t in range(NB):
            d = int(D[w, t])
            if d == 0:
                continue
            if cur and (curpos + d * 128 > MAXPOS or len(cur) >= MAXT):
                gw.append(cur)
                cur, curpos = [], 0
            cur.append(t)
            curpos += d * 128
        if cur:
            gw.append(cur)
        groups.append(gw)

    eorder = np.lexsort((loc, dstl, w_of, core))
    sc, sw, sd, sl = core[eorder], w_of[eorder], dstl[eorder], loc[eorder]
    key = ((sc * NW + sw) * RPC + sd).astype(np.int64)
    starts = np.searchsorted(key, np.arange(P * NW * RPC, dtype=np.int64))
    starts = np.append(starts, len(key))

    gidx_cores, sidx_cores = [], []
    for k in range(P):
        gparts, sparts = [], []
        for w in range(NW):
            od = order[k, w]
            for gt in groups[w]:
                for t in gt:
                    d = int(D[w, t])
                    nodes = od[t * 128:(t + 1) * 128]
                    blockg = np.full((d, 128), zloc_w[w], np.int32)
                    for p in range(128):
                        nloc = int(nodes[p])
                        s0 = starts[(k * NW + w) * RPC + nloc]
                        s1 = starts[(k * NW + w) * RPC + nloc + 1]
                        cnt = s1 - s0
                        if cnt:
                            blockg[:cnt, p] = sl[s0:s1]
                    gparts.append(blockg.reshape(-1))
                srows = np.concatenate(
                    [od[t * 128:(t + 1) * 128] for t in gt]).astype(np.int32)
                sparts.append(srows)
        gidx_cores.append(_wrap128(np.concatenate(gparts).astype(np.int16)))
        sidx_cores.append(_wrap128(np.concatenate(sparts).astype(np.int16)))
    return groups, D, gidx_cores, sidx_cores, order


def kernel(x, edge_index, edge_weight, emb, Wl1, bl1, Wr1, a1,
           Wl2, bl2, Wr2, a2, Wout, bout):
    import concourse.bacc as bacc
    import concourse.mybir as mybir
    import concourse.tile as tile
    from concourse.bass_utils import run_bass_kernel_spmd
    from concourse.masks import make_identity

    x = np.asarray(x).astype(np.int64)
    ei = np.asarray(edge_index).astype(np.int64)
    emb = np.asarray(emb, np.float32)
    Wl1 = np.asarray(Wl1, np.float32); Wr1 = np.asarray(Wr1, np.float32)
    Wl2 = np.asarray(Wl2, np.float32); Wr2 = np.asarray(Wr2, np.float32)
    Wout = np.asarray(Wout, np.float32)
    bl1 = np.asarray(bl1, np.float32); bl2 = np.asarray(bl2, np.float32)
    bout = np.asarray(bout, np.float32)
    a1f = float(np.asarray(a1)); a2f = float(np.asarray(a2))
    src, dst = ei[0], ei[1]

    # ---- host prep ------------------------------------------------------
    emb_hc = np.zeros((P * RPC, HID), np.float32)
    for r in range(P):
        emb_hc[r * RPC:r * RPC + NPC] = emb[r * NPC:(r + 1) * NPC]

    # per-core own h0 transposed (feature-major) [HID, RPC]
    h0T_own = np.zeros((P, HID, RPC), np.float32)
    for k in range(P):
        h0T_own[k, :, :NPC] = emb[x[k * NPC:(k + 1) * NPC]].T

    sid1 = x[src]
    w1 = sid1 // (2 * NPC)
    loc1 = RPC * ((sid1 // NPC) % 2) + sid1 % NPC
    g1, D1, gidx1, sidx1, ord1 = _build_layer_meta(w1, loc1, dst, [ZLOC] * NW)
    lr = src % NPC
    w2 = np.minimum(lr // 3200, 3)
    qr_a, qs_a = np.array(QR), np.array(QS)
    loc2 = (src // NPC) * qr_a[w2] + (lr - qs_a[w2])
    g2, D2, gidx2, sidx2, ord2 = _build_layer_meta(
        w2, loc2, dst, [P * QR[w] for w in range(NW)])

    # inverse counts (node order, [128, NB] partition-major)
    cnt = np.bincount(dst, minlength=N).astype(np.float32)
    invc = np.zeros((P, 128, NB), np.float32)
    for k in range(P):
        c = np.zeros(RPC, np.float32)
        c[:NPC] = 1.0 / np.maximum(cnt[k * NPC:(k + 1) * NPC], 1.0)
        invc[k] = c.reshape(NB, 128).T

    # ---- device program -------------------------------------------------
    f32, bf16, i16 = mybir.dt.float32, mybir.dt.bfloat16, mybir.dt.int16
    nc = bacc.Bacc(dynamic_dma_scratch_size=65536, num_swdge_queues=4)
    dp = nc.declare_dram_parameter
    embw = dp("embw", [P * RPC, HID], f32, isOutput=False)
    h0T_p = dp("h0T", [HID, RPC], f32, isOutput=False)
    gi1 = dp("gi1", list(gidx1[0].shape), i16, isOutput=False)
    si1 = dp("si1", list(sidx1[0].shape), i16, isOutput=False)
    gi2 = dp("gi2", list(gidx2[0].shape), i16, isOutput=False)
    si2 = dp("si2", list(sidx2[0].shape), i16, isOutput=False)
    invce_p = dp("invce", [128, NB, HID], f32, isOutput=False)
    wl1_p = dp("wl1", [HID, HID], f32, isOutput=False)
    wr1_p = dp("wr1", [HID, HID], f32, isOutput=False)
    wl2_p = dp("wl2", [HID, HID], f32, isOutput=False)
    wr2_p = dp("wr2", [HID, HID], f32, isOutput=False)
    wout_p = dp("wout", [HID, HID], f32, isOutput=False)
    bl1_p = dp("bl1t", [HID, 1], f32, isOutput=False)
    bl2_p = dp("bl2t", [HID, 1], f32, isOutput=False)
    bout_p = dp("boutr", [128, CH * HID], f32, isOutput=False)
    out_p = dp("out", [RPC, HID], f32, isOutput=True)

    acc_d = [[nc.dram_tensor(f"acc{li}_{w}", [ACCR, HID], f32)
              for w in range(NW)] for li in range(2)]
    hc1q = [nc.dram_tensor(f"hc1q{q}", [QR[q], HID], f32) for q in range(NW)]
    hcatq = [nc.dram_tensor(f"hcatq{q}", [P * QR[q] + 128, HID], f32,
                            addr_space="Shared") for q in range(NW)]

    AX = mybir.AxisListType.X
    ADD = mybir.AluOpType.add
    PRELU = mybir.ActivationFunctionType.Prelu

    qctr = [0]

    def next_q(ndesc):
        q = 1 + qctr[0] % 3
        qctr[0] += 1
        return q

    with tile.TileContext(nc) as tc:
        with tc.tile_pool(name="const", bufs=1) as cpool, \
             tc.tile_pool(name="big", bufs=1) as bpool, \
             tc.tile_pool(name="gio", bufs=3) as gpool, \
             tc.tile_pool(name="ph2", bufs=2) as qpool, \
             tc.tile_pool(name="ps", bufs=1, space="PSUM") as ppool:

            ident = cpool.tile([128, 128], f32)
            make_identity(nc, ident[:])
            ident_bf = cpool.tile([HID, HID], bf16)
            nc.vector.tensor_copy(ident_bf[:], ident[:HID, :HID])
            wl1_t = cpool.tile([HID, HID], f32); nc.sync.dma_start(wl1_t[:], wl1_p[:])
            wr1_t = cpool.tile([HID, HID], f32); nc.sync.dma_start(wr1_t[:], wr1_p[:])
            wl2_t = cpool.tile([HID, HID], f32); nc.sync.dma_start(wl2_t[:], wl2_p[:])
            wr2_t = cpool.tile([HID, HID], f32); nc.sync.dma_start(wr2_t[:], wr2_p[:])
            wr2b_t = cpool.tile([HID, HID], bf16)
            nc.vector.tensor_copy(wr2b_t[:], wr2_t[:])
            wout_t = cpool.tile([HID, HID], f32); nc.sync.dma_start(wout_t[:], wout_p[:])
            bl1_t = cpool.tile([HID, 1], f32); nc.sync.dma_start(bl1_t[:], bl1_p[:])
            bl2_t = cpool.tile([HID, 1], f32); nc.sync.dma_start(bl2_t[:], bl2_p[:])
            bout_t = cpool.tile([128, CH * HID], f32); nc.sync.dma_start(bout_t[:], bout_p[:])

            h1T = bpool.tile([HID, NB, 128], bf16)     # h1 transposed, own nodes
            zt = cpool.tile([128, HID], f32)
            nc.vector.memset(zt[:], 0.0)
            zbig = cpool.tile([128, 11 * HID], f32)
            nc.vector.memset(zbig[:], 0.0)

            def zero_accs(li):
                for w in range(NW):
                    # ACCR = 12672 = 128 * 99; partition p covers rows
                    # [99p, 99p+99) contiguously
                    dstv = acc_d[li][w][:].rearrange("(p b) f -> p (b f)", p=128)
                    for c in range(9):
                        nc.sync.dma_start(
                            dstv[:, c * 11 * HID:(c + 1) * 11 * HID], zbig[:])

            zero_accs(0)

            def phase1_setup(gi_p, si_p):
                gi_t = bpool.tile([128, gi_p.shape[1]], i16, tag="gi", name="gi_t")
                si_t = bpool.tile([128, si_p.shape[1]], i16, tag="si", name="si_t")
                nc.sync.dma_start(gi_t[:], gi_p[:])
                nc.sync.dma_start(si_t[:], si_p[:])
                return {"gi": gi_t, "si": si_t, "gcol": 0, "scol": 0}

            def phase1_window(st, li, groups, D, w, win):
                gi_t, si_t = st["gi"], st["si"]
                gcol, scol = st["gcol"], st["scol"]
                if True:
                    for gt in groups[w]:
                        npos = int(sum(D[w, t] for t in gt)) * 128
                        ncols = npos // 128
                        nt = len(gt)
                        g_t = gpool.tile([128, MAXPOS // 128, HID], f32, tag="g", name="g_t")
                        r_t = gpool.tile([128, MAXT, HID], f32, tag="r", name="r_t")
                        nc.gpsimd.dma_gather(
                            g_t[:, :ncols, :], win, gi_t[:, gcol:gcol + npos // 16],
                            npos, npos, HID, single_packet=False,
                            queue_num=next_q(npos))
                        off = 0
                        for i, t in enumerate(gt):
                            d = int(D[w, t])
                            view = g_t[:, off:off + d, :].rearrange("p d f -> p f d")
                            nc.vector.tensor_reduce(r_t[:, i, :], view, axis=AX, op=ADD)
                            off += d
                        nc.gpsimd.dma_scatter_add(
                            acc_d[li][w][:], r_t[:, :nt, :], si_t[:, scol:scol + nt * 8],
                            nt * 128, nt * 128, HID, single_packet=False,
                            queue_num=next_q(nt * 128))
                        gcol += npos // 16
                        scol += nt * 8
                st["gcol"], st["scol"] = gcol, scol

            def phase2(L, chunks=None):
                wl_t = wl1_t if L == 1 else wl2_t
                wr_t = wr1_t if L == 1 else wr2_t
                bl_t = bl1_t if L == 1 else bl2_t
                alpha = a1f if L == 1 else a2f
                if chunks is None:
                    chunks = [(c, min(CH, NB - c)) for c in range(0, NB, CH)]
                for c0, ct in chunks:
                    m4 = qpool.tile([128, NW, CH, HID], f32, tag="m4", name="m4")
                    for w in range(NW):
                        nc.scalar.dma_start(
                            m4[:, w, :ct, :],
                            acc_d[L - 1][w][c0 * 128:(c0 + ct) * 128]
                            .rearrange("(t p) f -> p t f", p=128))
                    invcc = qpool.tile([128, CH, HID], f32, tag="invcc", name="invcc")
                    nc.scalar.dma_start(invcc[:, :ct, :], invce_p[:, c0:c0 + ct, :])
                    mean0 = qpool.tile([128, CH, HID], f32, tag="mean0", name="mean0")
                    nc.vector.tensor_reduce(
                        mean0[:, :ct, :],
                        m4[:, :, :ct, :].rearrange("p w t f -> p t f w"),
                        axis=AX, op=ADD)
                    nc.vector.tensor_tensor(
                        mean0[:, :ct, :].rearrange("p t f -> p (t f)"),
                        mean0[:, :ct, :].rearrange("p t f -> p (t f)"),
                        invcc[:, :ct, :].rearrange("p t f -> p (t f)"),
                        op=mybir.AluOpType.mult)
                    # transpose ct tiles into psum (two banks of 4 tiles)
                    meanT = qpool.tile([HID, CH * 128], f32, tag="meanT", name="meanT")
                    for hb in range(0, ct, 4):
                        hn = min(4, ct - hb)
                        psT = ppool.tile([HID, 512], f32, tag=f"psT{(hb // 4) % 2}",
                                         name="psT")
                        for i in range(hn):
                            nc.tensor.transpose(
                                psT[:, i * 128:(i + 1) * 128],
                                mean0[:, hb + i, :], ident[:])
                        nc.vector.tensor_copy(
                            meanT[:, hb * 128:(hb + hn) * 128], psT[:, :hn * 128])
                    if L == 1:
                        hT = qpool.tile([HID, CH * 128], f32, tag="hT", name="hT")
                        nc.sync.dma_start(
                            hT[:, :ct * 128],
                            h0T_p[:, c0 * 128:(c0 + ct) * 128])
                    # matmuls in half-chunks of 4 tiles (psum 512 col limit)
                    for h in range(0, ct, 4):
                        hw = min(4, ct - h)
                        cols = slice(h * 128, (h + hw) * 128)
                        psC = ppool.tile([HID, 512], f32, tag=f"psC{(h // 4) % 2}",
                                         name="psC")
                        nc.tensor.matmul(psC[:, :hw * 128], wl_t[:], meanT[:, cols],
                                         start=True, stop=False)
                        if L == 1:
                            nc.tensor.matmul(psC[:, :hw * 128], wr_t[:], hT[:, cols],
                                             start=False, stop=True)
                        else:
                            nc.tensor.matmul(
                                psC[:, :hw * 128], wr2b_t[:],
                                h1T[:, c0 + h:c0 + h + hw, :]
                                .rearrange("f t n -> f (t n)"),
                                start=False, stop=True)
                        act_out = (h1T[:, c0 + h:c0 + h + hw, :] if L == 1 else
                                   h2T_t[:, h:h + hw, :])
                        nc.scalar.activation(
                            act_out.rearrange("f t n -> f (t n)"),
                            psC[:, :hw * 128], PRELU, bias=bl_t[:], alpha=alpha)
                    if L == 1:
                        # back-transpose to node-major and write hc1
                        psH = ppool.tile([128, CH * HID], bf16, tag="psH", name="psH")
                        for i in range(ct):
                            nc.tensor.transpose(
                                psH[:, i * HID:(i + 1) * HID],
                                h1T[:, c0 + i, :], ident_bf[:])
                        hc1c = qpool.tile([128, CH, HID], f32, tag="hc1c", name="hc1c")
                        nc.vector.tensor_copy(
                            hc1c[:, :ct, :].rearrange("p t f -> p (t f)"),
                            psH[:, :ct * HID])
                        # write into quarter tensors (split at boundaries)
                        r0, r1 = c0 * 128, (c0 + ct) * 128
                        for q in range(NW):
                            q0, q1 = QS[q], QS[q] + QR[q]
                            a, b = max(r0, q0), min(r1, q1)
                            if a >= b:
                                continue
                            t_a = (a - r0) // 128
                            t_b = (b - r0) // 128
                            nc.sync.dma_start(
                                hc1q[q][a - q0:b - q0]
                                .rearrange("(t p) f -> p t f", p=128),
                                hc1c[:, t_a:t_b, :])
                        if c0 + ct in (25, 50, 75):
                            qq = (c0 + ct) // 25 - 1
                            nc.gpsimd.collective_compute(
                                "AllGather", mybir.AluOpType.bypass,
                                replica_groups=[list(range(P))],
                                ins=[hc1q[qq][:]],
                                outs=[hcatq[qq][:P * QR[qq]]])
                    else:
                        psE = ppool.tile([128, CH * HID], f32, tag="psE", name="psE")
                        for i in range(ct):
                            nc.tensor.matmul(
                                psE[:, i * HID:(i + 1) * HID],
                                h2T_t[:, i, :], wout_t[:], start=True, stop=True)
                        outc = qpool.tile([128, CH, HID], f32, tag="outc", name="outc")
                        nc.vector.tensor_tensor(
                            outc[:, :ct, :].rearrange("p t f -> p (t f)"),
                            psE[:, :ct * HID], bout_t[:, :ct * HID], op=ADD)
                        nc.sync.dma_start(
                            out_p[c0 * 128:(c0 + ct) * 128]
                            .rearrange("(t p) f -> p t f", p=128),
                            outc[:, :ct, :])

            h2T_t = bpool.tile([HID, CH, 128], f32)

            # zero the per-window pad blocks of the shared gather sources
            for q in range(NW):
                nc.sync.dma_start(hcatq[q][P * QR[q]:], zt[:])
            # ---- layer 1 ----
            st1 = phase1_setup(gi1, si1)
            for w in range(NW):
                phase1_window(st1, 0, g1, D1, w, embw[w * WIN:(w + 1) * WIN])
            zero_accs(1)
            phase2(1, chunks=[(0, 8), (8, 8), (16, 8), (24, 1),
                              (25, 8), (33, 8), (41, 8), (49, 1),
                              (50, 8), (58, 8), (66, 8), (74, 1),
                              (75, 8), (83, 8), (91, 7)])
            # zero pad rows of h1 (nodes 12500..12543) so gather pads stay 0
            nc.sync.dma_start(hc1q[3][NPC - QS[3]:], zt[:RPC - NPC, :])
            nc.gpsimd.collective_compute(
                "AllGather", mybir.AluOpType.bypass,
                replica_groups=[list(range(P))],
                ins=[hc1q[3][:]],
                outs=[hcatq[3][:P * QR[3]]])
            # ---- layer 2 + out ----
            st2 = phase1_setup(gi2, si2)
            for w in range(NW):
                phase1_window(st2, 1, g2, D2, w, hcatq[w][:])
            phase2(2)

    nc.compile()

    in_maps = []
    for k in range(P):
        in_maps.append({
            "embw": emb_hc, "h0T": h0T_own[k],
            "gi1": gidx1[k], "si1": sidx1[k],
            "gi2": gidx2[k], "si2": sidx2[k],
            "invce": np.repeat(invc[k].reshape(128, NB, 1), HID, axis=2),
            "wl1": Wl1, "wr1": Wr1, "wl2": Wl2, "wr2": Wr2, "wout": Wout,
            "bl1t": bl1.reshape(HID, 1), "bl2t": bl2.reshape(HID, 1),
            "boutr": np.tile(bout.reshape(1, HID), (128, CH)),
        })
    res = run_bass_kernel_spmd(nc, in_maps, list(range(P)))
    out = np.zeros((N, HID), np.float32)
    for k in range(P):
        out[k * NPC:(k + 1) * NPC] = res.results[k]["out"][:NPC]
    kernel.last_exec_time_ns = res.exec_time_ns
    return out


# revision 19
# speedup vs baseline: 1.4449x; 1.1429x over previous
"""GNN (2x SAGEConv + linear) Bass kernel for trn2, 8 NeuronCores.

Sharding: nodes partitioned across 8 cores (12500 each, dst-range).
Each layer: per-core windowed padded-CSR gathers of h[src] (dma_gather,
int16 windows of 25088 hcat rows), on-chip segment reduce (DVE strided),
batched unique-row dma_scatter_add into per-window DRAM accumulators,
dense combine + PE MLP.  One AllGather of h1 slices between layers.

v3: SWDGE queue rotation (4 Q7 core pairs in parallel for gather/scatter
descriptor generation), contiguous accumulator zeroing, 1/cnt folded into
phase-1 scatter, chunk-batched phase 2 (fat DVE/PE ops), feature-major
host-prepped h0T, bf16 h1T.
"""
import numpy as np

N = 100000
E = 1250000
HID = 64
P = 8
NPC = 12500          # nodes per core
RPC = 12544          # rows per core block (98 * 128), rows 12500+ are zero pads
NB = RPC // 128      # 98 blocks
WIN = 2 * RPC        # 25088 rows per gather window (2 rank blocks)
NW = 4               # windows
ZLOC = 12500         # local row inside a window that is guaranteed zero
ACCR = RPC + 128     # accumulator rows (tail rows are scratch)
MAXPOS = 4096        # max gather positions per call
MAXT = 16            # max tiles per gather call
CH = 8               # phase-2 tiles per chunk
QR = [3200, 3200, 3200, 2944]   # hc1 quarter rows (128-aligned tiles)
QS = [0, 3200, 6400, 9600]      # quarter start rows


def _wrap128(vals):
    """flat int16 stream -> [128, len/16] wrapped+replicated layout."""
    n = vals.shape[0]
    w16 = np.ascontiguousarray(vals.reshape(n // 16, 16).T)
    return np.tile(w16, (8, 1))


def _build_layer_meta(w_of, loc, dst, zloc_w):
    """Per-layer gather/scatter metadata (see v2 docstring)."""
    core = dst // NPC
    dstl = dst % NPC

    deg = np.zeros((P, NW, RPC), np.int32)
    np.add.at(deg, (core, w_of, dstl), 1)
    order = np.argsort(-deg, axis=2, kind="stable")  # [P, NW, RPC]
    deg_sorted = -np.sort(-deg, axis=2)
    tile_max = deg_sorted.reshape(P, NW, NB, 128).max(axis=3)
    D = tile_max.max(axis=0)                         # [NW, NB] shared

    groups = []
    for w in range(NW):
        gw = []
        cur, curpos = [], 0
        for t in range(NB):
            d = int(D[w, t])
            if d == 0:
                continue
            if cur and (curpos + d * 128 > MAXPOS or len(cur) >= MAXT):
                gw.append(cur)
                cur, curpos = [], 0
            cur.append(t)
            curpos += d * 128
        if cur:
            gw.append(cur)
        groups.append(gw)

    eorder = np.lexsort((loc, dstl, w_of, core))
    sc, sw, sd, sl = core[eorder], w_of[eorder], dstl[eorder], loc[eorder]
    key = ((sc * NW + sw) * RPC + sd).astype(np.int64)
    starts = np.searchsorted(key, np.arange(P * NW * RPC, dtype=np.int64))
    starts = np.append(starts, len(key))

    gidx_cores, sidx_cores = [], []
    for k in range(P):
        gparts, sparts = [], []
        for w in range(NW):
            od = order[k, w]
            for gt in groups[w]:
                for t in gt:
                    d = int(D[w, t])
                    nodes = od[t * 128:(t + 1) * 128]
                    blockg = np.full((d, 128), zloc_w[w], np.int32)
                    for p in range(128):
                        nloc = int(nodes[p])
                        s0 = starts[(k * NW + w) * RPC + nloc]
                        s1 = starts[(k * NW + w) * RPC + nloc + 1]
                        cnt = s1 - s0
                        if cnt:
                            blockg[:cnt, p] = sl[s0:s1]
                    gparts.append(blockg.reshape(-1))
                srows = np.concatenate(
                    [od[t * 128:(t + 1) * 128] for t in gt]).astype(np.int32)
                sparts.append(srows)
        gidx_cores.append(_wrap128(np.concatenate(gparts).astype(np.int16)))
        sidx_cores.append(_wrap128(np.concatenate(sparts).astype(np.int16)))
    return groups, D, gidx_cores, sidx_cores, order


def kernel(x, edge_index, edge_weight, emb, Wl1, bl1, Wr1, a1,
           Wl2, bl2, Wr2, a2, Wout, bout):
    import concourse.bacc as bacc
    import concourse.mybir as mybir
    import concourse.tile as tile
    from concourse.bass_utils import run_bass_kernel_spmd
    from concourse.masks import make_identity

    x = np.asarray(x).astype(np.int64)
    ei = np.asarray(edge_index).astype(np.int64)
    emb = np.asarray(emb, np.float32)
    Wl1 = np.asarray(Wl1, np.float32); Wr1 = np.asarray(Wr1, np.float32)
    Wl2 = np.asarray(Wl2, np.float32); Wr2 = np.asarray(Wr2, np.float32)
    Wout = np.asarray(Wout, np.float32)
    bl1 = np.asarray(bl1, np.float32); bl2 = np.asarray(bl2, np.float32)
    bout = np.asarray(bout, np.float32)
    a1f = float(np.asarray(a1)); a2f = float(np.asarray(a2))
    src, dst = ei[0], ei[1]

    # ---- host prep ------------------------------------------------------
    emb_hc = np.zeros((P * RPC, HID), np.float32)
    for r in range(P):
        emb_hc[r * RPC:r * RPC + NPC] = emb[r * NPC:(r + 1) * NPC]

    # per-core own h0 transposed (feature-major) [HID, RPC]
    h0T_own = np.zeros((P, HID, RPC), np.float32)
    for k in range(P):
        h0T_own[k, :, :NPC] = emb[x[k * NPC:(k + 1) * NPC]].T

    sid1 = x[src]
    w1 = sid1 // (2 * NPC)
    loc1 = RPC * ((sid1 // NPC) % 2) + sid1 % NPC
    g1, D1, gidx1, sidx1, ord1 = _build_layer_meta(w1, loc1, dst, [ZLOC] * NW)
    lr = src % NPC
    w2 = np.minimum(lr // 3200, 3)
    qr_a, qs_a = np.array(QR), np.array(QS)
    loc2 = (src // NPC) * qr_a[w2] + (lr - qs_a[w2])
    g2, D2, gidx2, sidx2, ord2 = _build_layer_meta(
        w2, loc2, dst, [P * QR[w] for w in range(NW)])

    # inverse counts (node order, [128, NB] partition-major)
    cnt = np.bincount(dst, minlength=N).astype(np.float32)
    invc = np.zeros((P, 128, NB), np.float32)
    for k in range(P):
        c = np.zeros(RPC, np.float32)
        c[:NPC] = 1.0 / np.maximum(cnt[k * NPC:(k + 1) * NPC], 1.0)
        invc[k] = c.reshape(NB, 128).T

    # ---- device program -------------------------------------------------
    f32, bf16, i16 = mybir.dt.float32, mybir.dt.bfloat16, mybir.dt.int16
    nc = bacc.Bacc(dynamic_dma_scratch_size=65536, num_swdge_queues=4)
    dp = nc.declare_dram_parameter
    embw = dp("embw", [P * RPC, HID], f32, isOutput=False)
    h0T_p = dp("h0T", [HID, RPC], f32, isOutput=False)
    gi1 = dp("gi1", list(gidx1[0].shape), i16, isOutput=False)
    si1 = dp("si1", list(sidx1[0].shape), i16, isOutput=False)
    gi2 = dp("gi2", list(gidx2[0].shape), i16, isOutput=False)
    si2 = dp("si2", list(sidx2[0].shape), i16, isOutput=False)
    invce_p = dp("invce", [128, NB, HID], f32, isOutput=False)
    wl1_p = dp("wl1", [HID, HID], f32, isOutput=False)
    wr1_p = dp("wr1", [HID, HID], f32, isOutput=False)
    wl2_p = dp("wl2", [HID, HID], f32, isOutput=False)
    wr2_p = dp("wr2", [HID, HID], f32, isOutput=False)
    wout_p = dp("wout", [HID, HID], f32, isOutput=False)
    bl1_p = dp("bl1t", [HID, 1], f32, isOutput=False)
    bl2_p = dp("bl2t", [HID, 1], f32, isOutput=False)
    bout_p = dp("boutr", [128, CH * HID], f32, isOutput=False)
    out_p = dp("out", [RPC, HID], f32, isOutput=True)

    acc_d = [[nc.dram_tensor(f"acc{li}_{w}", [ACCR, HID], f32)
              for w in range(NW)] for li in range(2)]
    hc1q = [nc.dram_tensor(f"hc1q{q}", [QR[q], HID], f32) for q in range(NW)]
    hcatq = [nc.dram_tensor(f"hcatq{q}", [P * QR[q] + 128, HID], f32,
                            addr_space="Shared") for q in range(NW)]

    AX = mybir.AxisListType.X
    ADD = mybir.AluOpType.add
    PRELU = mybir.ActivationFunctionType.Prelu

    qctr = [0]

    def next_q(ndesc):
        q = 1 + qctr[0] % 3
        qctr[0] += 1
        return q

    with tile.TileContext(nc) as tc:
        with tc.tile_pool(name="const", bufs=1) as cpool, \
             tc.tile_pool(name="big", bufs=1) as bpool, \
             tc.tile_pool(name="gio", bufs=3) as gpool, \
             tc.tile_pool(name="ph2", bufs=2) as qpool, \
             tc.tile_pool(name="ps", bufs=1, space="PSUM") as ppool:

            ident = cpool.tile([128, 128], f32)
            make_identity(nc, ident[:])
            ident_bf = cpool.tile([HID, HID], bf16)
            nc.vector.tensor_copy(ident_bf[:], ident[:HID, :HID])
            wl1_t = cpool.tile([HID, HID], f32); nc.sync.dma_start(wl1_t[:], wl1_p[:])
            wr1_t = cpool.tile([HID, HID], f32); nc.sync.dma_start(wr1_t[:], wr1_p[:])
            wl2_t = cpool.tile([HID, HID], f32); nc.sync.dma_start(wl2_t[:], wl2_p[:])
            wr2_t = cpool.tile([HID, HID], f32); nc.sync.dma_start(wr2_t[:], wr2_p[:])
            wr2b_t = cpool.tile([HID, HID], bf16)
            nc.vector.tensor_copy(wr2b_t[:], wr2_t[:])
            wout_t = cpool.tile([HID, HID], f32); nc.sync.dma_start(wout_t[:], wout_p[:])
            bl1_t = cpool.tile([HID, 1], f32); nc.sync.dma_start(bl1_t[:], bl1_p[:])
            bl2_t = cpool.tile([HID, 1], f32); nc.sync.dma_start(bl2_t[:], bl2_p[:])
            bout_t = cpool.tile([128, CH * HID], f32); nc.sync.dma_start(bout_t[:], bout_p[:])

            h1T = bpool.tile([HID, NB, 128], bf16)     # h1 transposed, own nodes
            zt = cpool.tile([128, HID], f32)
            nc.vector.memset(zt[:], 0.0)
            zbig = cpool.tile([128, 11 * HID], f32)
            nc.vector.memset(zbig[:], 0.0)

            def zero_accs(li):
                for w in range(NW):
                    # ACCR = 12672 = 128 * 99; partition p covers rows
                    # [99p, 99p+99) contiguously
                    dstv = acc_d[li][w][:].rearrange("(p b) f -> p (b f)", p=128)
                    for c in range(9):
                        nc.sync.dma_start(
                            dstv[:, c * 11 * HID:(c + 1) * 11 * HID], zbig[:])

            zero_accs(0)

            def phase1_setup(gi_p, si_p):
                gi_t = bpool.tile([128, gi_p.shape[1]], i16, tag="gi", name="gi_t")
                si_t = bpool.tile([128, si_p.shape[1]], i16, tag="si", name="si_t")
                nc.sync.dma_start(gi_t[:], gi_p[:])
                nc.sync.dma_start(si_t[:], si_p[:])
                return {"gi": gi_t, "si": si_t, "gcol": 0, "scol": 0}

            def phase1_window(st, li, groups, D, w, win, depth=2):
                gi_t, si_t = st["gi"], st["si"]
                gcol, scol = st["gcol"], st["scol"]
                pend = []

                def flush_one():
                    r_p, nt_p, sc_p = pend.pop(0)
                    nc.gpsimd.dma_scatter_add(
                        acc_d[li][w][:], r_p[:, :nt_p, :],
                        si_t[:, sc_p:sc_p + nt_p * 8],
                        nt_p * 128, nt_p * 128, HID, single_packet=False,
                        queue_num=next_q(nt_p * 128))

                for gt in groups[w]:
                    npos = int(sum(D[w, t] for t in gt)) * 128
                    ncols = npos // 128
                    nt = len(gt)
                    g_t = gpool.tile([128, MAXPOS // 128, HID], f32, tag="g", name="g_t")
                    r_t = gpool.tile([128, MAXT, HID], f32, tag="r", name="r_t")
                    nc.gpsimd.dma_gather(
                        g_t[:, :ncols, :], win, gi_t[:, gcol:gcol + npos // 16],
                        npos, npos, HID, single_packet=False,
                        queue_num=next_q(npos))
                    off = 0
                    for i, t in enumerate(gt):
                        d = int(D[w, t])
                        view = g_t[:, off:off + d, :].rearrange("p d f -> p f d")
                        nc.vector.tensor_reduce(r_t[:, i, :], view, axis=AX, op=ADD)
                        off += d
                    pend.append((r_t, nt, scol))
                    gcol += npos // 16
                    scol += nt * 8
                    if len(pend) > depth:
                        flush_one()
                while pend:
                    flush_one()
                st["gcol"], st["scol"] = gcol, scol

            def phase2(L, chunks=None):
                wl_t = wl1_t if L == 1 else wl2_t
                wr_t = wr1_t if L == 1 else wr2_t
                bl_t = bl1_t if L == 1 else bl2_t
                alpha = a1f if L == 1 else a2f
                if chunks is None:
                    chunks = [(c, min(CH, NB - c)) for c in range(0, NB, CH)]
                for c0, ct in chunks:
                    m4 = qpool.tile([128, NW, CH, HID], f32, tag="m4", name="m4")
                    for w in range(NW):
                        nc.scalar.dma_start(
                            m4[:, w, :ct, :],
                            acc_d[L - 1][w][c0 * 128:(c0 + ct) * 128]
                            .rearrange("(t p) f -> p t f", p=128))
                    invcc = qpool.tile([128, CH, HID], f32, tag="invcc", name="invcc")
                    nc.scalar.dma_start(invcc[:, :ct, :], invce_p[:, c0:c0 + ct, :])
                    mean0 = qpool.tile([128, CH, HID], f32, tag="mean0", name="mean0")
                    nc.vector.tensor_reduce(
                        mean0[:, :ct, :],
                        m4[:, :, :ct, :].rearrange("p w t f -> p t f w"),
                        axis=AX, op=ADD)
                    nc.vector.tensor_tensor(
                        mean0[:, :ct, :].rearrange("p t f -> p (t f)"),
                        mean0[:, :ct, :].rearrange("p t f -> p (t f)"),
                        invcc[:, :ct, :].rearrange("p t f -> p (t f)"),
                        op=mybir.AluOpType.mult)
                    # transpose ct tiles into psum (two banks of 4 tiles)
                    meanT = qpool.tile([HID, CH * 128], f32, tag="meanT", name="meanT")
                    for hb in range(0, ct, 4):
                        hn = min(4, ct - hb)
                        psT = ppool.tile([HID, 512], f32, tag=f"psT{(hb // 4) % 2}",
                                         name="psT")
                        for i in range(hn):
                            nc.tensor.transpose(
                                psT[:, i * 128:(i + 1) * 128],
                                mean0[:, hb + i, :], ident[:])
                        nc.vector.tensor_copy(
                            meanT[:, hb * 128:(hb + hn) * 128], psT[:, :hn * 128])
                    if L == 1:
                        hT = qpool.tile([HID, CH * 128], f32, tag="hT", name="hT")
                        nc.sync.dma_start(
                            hT[:, :ct * 128],
                            h0T_p[:, c0 * 128:(c0 + ct) * 128])
                    # matmuls in half-chunks of 4 tiles (psum 512 col limit)
                    for h in range(0, ct, 4):
                        hw = min(4, ct - h)
                        cols = slice(h * 128, (h + hw) * 128)
                        psC = ppool.tile([HID, 512], f32, tag=f"psC{(h // 4) % 2}",
                                         name="psC")
                        nc.tensor.matmul(psC[:, :hw * 128], wl_t[:], meanT[:, cols],
                                         start=True, stop=False)
                        if L == 1:
                            nc.tensor.matmul(psC[:, :hw * 128], wr_t[:], hT[:, cols],
                                             start=False, stop=True)
                        else:
                            nc.tensor.matmul(
                                psC[:, :hw * 128], wr2b_t[:],
                                h1T[:, c0 + h:c0 + h + hw, :]
                                .rearrange("f t n -> f (t n)"),
                                start=False, stop=True)
                        act_out = (h1T[:, c0 + h:c0 + h + hw, :] if L == 1 else
                                   h2T_t[:, h:h + hw, :])
                        nc.scalar.activation(
                            act_out.rearrange("f t n -> f (t n)"),
                            psC[:, :hw * 128], PRELU, bias=bl_t[:], alpha=alpha)
                    if L == 1:
                        # back-transpose to node-major and write hc1
                        psH = ppool.tile([128, CH * HID], bf16, tag="psH", name="psH")
                        for i in range(ct):
                            nc.tensor.transpose(
                                psH[:, i * HID:(i + 1) * HID],
                                h1T[:, c0 + i, :], ident_bf[:])
                        hc1c = qpool.tile([128, CH, HID], f32, tag="hc1c", name="hc1c")
                        nc.vector.tensor_copy(
                            hc1c[:, :ct, :].rearrange("p t f -> p (t f)"),
                            psH[:, :ct * HID])
                        # write into quarter tensors (split at boundaries)
                        r0, r1 = c0 * 128, (c0 + ct) * 128
                        for q in range(NW):
                            q0, q1 = QS[q], QS[q] + QR[q]
                            a, b = max(r0, q0), min(r1, q1)
                            if a >= b:
                                continue
                            t_a = (a - r0) // 128
                            t_b = (b - r0) // 128
                            nc.sync.dma_start(
                                hc1q[q][a - q0:b - q0]
                                .rearrange("(t p) f -> p t f", p=128),
                                hc1c[:, t_a:t_b, :])
                        if c0 + ct in (25, 50, 75):
                            qq = (c0 + ct) // 25 - 1
                            nc.gpsimd.collective_compute(
                                "AllGather", mybir.AluOpType.bypass,
                                replica_groups=[list(range(P))],
                                ins=[hc1q[qq][:]],
                                outs=[hcatq[qq][:P * QR[qq]]])
                    else:
                        psE = ppool.tile([128, CH * HID], f32, tag="psE", name="psE")
                        for i in range(ct):
                            nc.tensor.matmul(
                                psE[:, i * HID:(i + 1) * HID],
                                h2T_t[:, i, :], wout_t[:], start=True, stop=True)
                        outc = qpool.tile([128, CH, HID], f32, tag="outc", name="outc")
                        nc.vector.tensor_tensor(
                            outc[:, :ct, :].rearrange("p t f -> p (t f)"),
                            psE[:, :ct * HID], bout_t[:, :ct * HID], op=ADD)
                        nc.sync.dma_start(
                            out_p[c0 * 128:(c0 + ct) * 128]
                            .rearrange("(t p) f -> p t f", p=128),
                            outc[:, :ct, :])

            h2T_t = bpool.tile([HID, CH, 128], f32)

            # zero the per-window pad blocks of the shared gather sources
            for q in range(NW):
                nc.sync.dma_start(hcatq[q][P * QR[q]:], zt[:])
            # ---- layer 1 ----
            st1 = phase1_setup(gi1, si1)
            for w in range(NW):
                phase1_window(st1, 0, g1, D1, w, embw[w * WIN:(w + 1) * WIN])
            zero_accs(1)
            phase2(1, chunks=[(0, 8), (8, 8), (16, 8), (24, 1),
                              (25, 8), (33, 8), (41, 8), (49, 1),
                              (50, 8), (58, 8), (66, 8), (74, 1),
                              (75, 8), (83, 8), (91, 7)])
            # zero pad rows of h1 (nodes 12500..12543) so gather pads stay 0
            nc.sync.dma_start(hc1q[3][NPC - QS[3]:], zt[:RPC - NPC, :])
            nc.gpsimd.collective_compute(
                "AllGather", mybir.AluOpType.bypass,
                replica_groups=[list(range(P))],
                ins=[hc1q[3][:]],
                outs=[hcatq[3][:P * QR[3]]])
            # ---- layer 2 + out ----
            st2 = phase1_setup(gi2, si2)
            for w in range(NW):
                phase1_window(st2, 1, g2, D2, w, hcatq[w][:])
            phase2(2)

    nc.compile()

    in_maps = []
    for k in range(P):
        in_maps.append({
            "embw": emb_hc, "h0T": h0T_own[k],
            "gi1": gidx1[k], "si1": sidx1[k],
            "gi2": gidx2[k], "si2": sidx2[k],
            "invce": np.repeat(invc[k].reshape(128, NB, 1), HID, axis=2),
            "wl1": Wl1, "wr1": Wr1, "wl2": Wl2, "wr2": Wr2, "wout": Wout,
            "bl1t": bl1.reshape(HID, 1), "bl2t": bl2.reshape(HID, 1),
            "boutr": np.tile(bout.reshape(1, HID), (128, CH)),
        })
    res = run_bass_kernel_spmd(nc, in_maps, list(range(P)))
    out = np.zeros((N, HID), np.float32)
    for k in range(P):
        out[k * NPC:(k + 1) * NPC] = res.results[k]["out"][:NPC]
    kernel.last_exec_time_ns = res.exec_time_ns
    return out


# revision 20
# speedup vs baseline: 1.4869x; 1.0291x over previous
"""GNN (2x SAGEConv + linear) Bass kernel for trn2, 8 NeuronCores.

Sharding: nodes partitioned across 8 cores (12500 each, dst-range).
Each layer: per-core windowed padded-CSR gathers of h[src] (dma_gather,
int16 windows of 25088 hcat rows), on-chip segment reduce (DVE strided),
batched unique-row dma_scatter_add into per-window DRAM accumulators,
dense combine + PE MLP.  One AllGather of h1 slices between layers.

v3: SWDGE queue rotation (4 Q7 core pairs in parallel for gather/scatter
descriptor generation), contiguous accumulator zeroing, 1/cnt folded into
phase-1 scatter, chunk-batched phase 2 (fat DVE/PE ops), feature-major
host-prepped h0T, bf16 h1T.
"""
import numpy as np

N = 100000
E = 1250000
HID = 64
P = 8
NPC = 12500          # nodes per core
RPC = 12544          # rows per core block (98 * 128), rows 12500+ are zero pads
NB = RPC // 128      # 98 blocks
WIN = 2 * RPC        # 25088 rows per gather window (2 rank blocks)
NW = 4               # windows
ZLOC = 12500         # local row inside a window that is guaranteed zero
ACCR = RPC + 128     # accumulator rows (tail rows are scratch)
MAXPOS = 4096        # max gather positions per call
MAXT = 16            # max tiles per gather call
CH = 8               # phase-2 tiles per chunk
QR = [3200, 3200, 3200, 2944]   # hc1 quarter rows (128-aligned tiles)
QS = [0, 3200, 6400, 9600]      # quarter start rows


def _wrap128(vals):
    """flat int16 stream -> [128, len/16] wrapped+replicated layout."""
    n = vals.shape[0]
    w16 = np.ascontiguousarray(vals.reshape(n // 16, 16).T)
    return np.tile(w16, (8, 1))


def _build_layer_meta(w_of, loc, dst, zloc_w):
    """Per-layer gather/scatter metadata (see v2 docstring)."""
    core = dst // NPC
    dstl = dst % NPC

    deg = np.zeros((P, NW, RPC), np.int32)
    np.add.at(deg, (core, w_of, dstl), 1)
    order = np.argsort(-deg, axis=2, kind="stable")  # [P, NW, RPC]
    deg_sorted = -np.sort(-deg, axis=2)
    tile_max = deg_sorted.reshape(P, NW, NB, 128).max(axis=3)
    D = tile_max.max(axis=0)                         # [NW, NB] shared

    groups = []
    for w in range(NW):
        gw = []
        cur, curpos = [], 0
        for t in range(NB):
            d = int(D[w, t])
            if d == 0:
                continue
            if cur and (curpos + d * 128 > MAXPOS or len(cur) >= MAXT):
                gw.append(cur)
                cur, curpos = [], 0
            cur.append(t)
            curpos += d * 128
        if cur:
            gw.append(cur)
        groups.append(gw)

    eorder = np.lexsort((loc, dstl, w_of, core))
    sc, sw, sd, sl = core[eorder], w_of[eorder], dstl[eorder], loc[eorder]
    key = ((sc * NW + sw) * RPC + sd).astype(np.int64)
    starts = np.searchsorted(key, np.arange(P * NW * RPC, dtype=np.int64))
    starts = np.append(starts, len(key))

    gidx_cores, sidx_cores = [], []
    for k in range(P):
        gparts, sparts = [], []
        for w in range(NW):
            od = order[k, w]
            for gt in groups[w]:
                for t in gt:
                    d = int(D[w, t])
                    nodes = od[t * 128:(t + 1) * 128]
                    blockg = np.full((d, 128), zloc_w[w], np.int32)
                    for p in range(128):
                        nloc = int(nodes[p])
                        s0 = starts[(k * NW + w) * RPC + nloc]
                        s1 = starts[(k * NW + w) * RPC + nloc + 1]
                        cnt = s1 - s0
                        if cnt:
                            blockg[:cnt, p] = sl[s0:s1]
                    gparts.append(blockg.reshape(-1))
                srows = np.concatenate(
                    [od[t * 128:(t + 1) * 128] for t in gt]).astype(np.int32)
                sparts.append(srows)
        gidx_cores.append(_wrap128(np.concatenate(gparts).astype(np.int16)))
        sidx_cores.append(_wrap128(np.concatenate(sparts).astype(np.int16)))
    return groups, D, gidx_cores, sidx_cores, order


def kernel(x, edge_index, edge_weight, emb, Wl1, bl1, Wr1, a1,
           Wl2, bl2, Wr2, a2, Wout, bout):
    import concourse.bacc as bacc
    import concourse.mybir as mybir
    import concourse.tile as tile
    from concourse.bass_utils import run_bass_kernel_spmd
    from concourse.masks import make_identity

    x = np.asarray(x).astype(np.int64)
    ei = np.asarray(edge_index).astype(np.int64)
    emb = np.asarray(emb, np.float32)
    Wl1 = np.asarray(Wl1, np.float32); Wr1 = np.asarray(Wr1, np.float32)
    Wl2 = np.asarray(Wl2, np.float32); Wr2 = np.asarray(Wr2, np.float32)
    Wout = np.asarray(Wout, np.float32)
    bl1 = np.asarray(bl1, np.float32); bl2 = np.asarray(bl2, np.float32)
    bout = np.asarray(bout, np.float32)
    a1f = float(np.asarray(a1)); a2f = float(np.asarray(a2))
    src, dst = ei[0], ei[1]

    # ---- host prep ------------------------------------------------------
    emb_hc = np.zeros((P * RPC, HID), np.float32)
    for r in range(P):
        emb_hc[r * RPC:r * RPC + NPC] = emb[r * NPC:(r + 1) * NPC]

    # per-core own h0 transposed (feature-major) [HID, RPC]
    h0T_own = np.zeros((P, HID, RPC), np.float32)
    for k in range(P):
        h0T_own[k, :, :NPC] = emb[x[k * NPC:(k + 1) * NPC]].T

    sid1 = x[src]
    w1 = sid1 // (2 * NPC)
    loc1 = RPC * ((sid1 // NPC) % 2) + sid1 % NPC
    g1, D1, gidx1, sidx1, ord1 = _build_layer_meta(w1, loc1, dst, [ZLOC] * NW)
    lr = src % NPC
    w2 = np.minimum(lr // 3200, 3)
    qr_a, qs_a = np.array(QR), np.array(QS)
    loc2 = (src // NPC) * qr_a[w2] + (lr - qs_a[w2])
    g2, D2, gidx2, sidx2, ord2 = _build_layer_meta(
        w2, loc2, dst, [P * QR[w] for w in range(NW)])

    # inverse counts (node order, [128, NB] partition-major)
    cnt = np.bincount(dst, minlength=N).astype(np.float32)
    invc = np.zeros((P, 128, NB), np.float32)
    for k in range(P):
        c = np.zeros(RPC, np.float32)
        c[:NPC] = 1.0 / np.maximum(cnt[k * NPC:(k + 1) * NPC], 1.0)
        invc[k] = c.reshape(NB, 128).T

    # ---- device program -------------------------------------------------
    f32, bf16, i16 = mybir.dt.float32, mybir.dt.bfloat16, mybir.dt.int16
    nc = bacc.Bacc(dynamic_dma_scratch_size=65536, num_swdge_queues=4)
    dp = nc.declare_dram_parameter
    embw = dp("embw", [P * RPC, HID], f32, isOutput=False)
    h0T_p = dp("h0T", [HID, RPC], f32, isOutput=False)
    gi1 = dp("gi1", list(gidx1[0].shape), i16, isOutput=False)
    si1 = dp("si1", list(sidx1[0].shape), i16, isOutput=False)
    gi2 = dp("gi2", list(gidx2[0].shape), i16, isOutput=False)
    si2 = dp("si2", list(sidx2[0].shape), i16, isOutput=False)
    invce_p = dp("invce", [128, NB, HID], f32, isOutput=False)
    wl1_p = dp("wl1", [HID, HID], f32, isOutput=False)
    wr1_p = dp("wr1", [HID, HID], f32, isOutput=False)
    wl2_p = dp("wl2", [HID, HID], f32, isOutput=False)
    wr2_p = dp("wr2", [HID, HID], f32, isOutput=False)
    wout_p = dp("wout", [HID, HID], f32, isOutput=False)
    bl1_p = dp("bl1t", [HID, 1], f32, isOutput=False)
    bl2_p = dp("bl2t", [HID, 1], f32, isOutput=False)
    bout_p = dp("boutr", [128, CH * HID], f32, isOutput=False)
    out_p = dp("out", [RPC, HID], f32, isOutput=True)

    acc_d = [[nc.dram_tensor(f"acc{li}_{w}", [ACCR, HID], f32)
              for w in range(NW)] for li in range(2)]
    hc1q = [nc.dram_tensor(f"hc1q{q}", [QR[q], HID], f32) for q in range(NW)]
    hcatq = [nc.dram_tensor(f"hcatq{q}", [P * QR[q] + 128, HID], f32,
                            addr_space="Shared") for q in range(NW)]

    AX = mybir.AxisListType.X
    ADD = mybir.AluOpType.add
    PRELU = mybir.ActivationFunctionType.Prelu

    qctr = [0]

    def next_q(ndesc):
        q = 1 + qctr[0] % 3
        qctr[0] += 1
        return q

    with tile.TileContext(nc) as tc:
        with tc.tile_pool(name="const", bufs=1) as cpool, \
             tc.tile_pool(name="big", bufs=1) as bpool, \
             tc.tile_pool(name="gio", bufs=3) as gpool, \
             tc.tile_pool(name="ph2", bufs=2) as qpool, \
             tc.tile_pool(name="ps", bufs=1, space="PSUM") as ppool:

            ident = cpool.tile([128, 128], f32)
            make_identity(nc, ident[:])
            ident_bf = cpool.tile([HID, HID], bf16)
            nc.vector.tensor_copy(ident_bf[:], ident[:HID, :HID])
            wl1_t = cpool.tile([HID, HID], f32); nc.sync.dma_start(wl1_t[:], wl1_p[:])
            wr1_t = cpool.tile([HID, HID], f32); nc.sync.dma_start(wr1_t[:], wr1_p[:])
            wl2_t = cpool.tile([HID, HID], f32); nc.sync.dma_start(wl2_t[:], wl2_p[:])
            wr2_t = cpool.tile([HID, HID], f32); nc.sync.dma_start(wr2_t[:], wr2_p[:])
            wr2b_t = cpool.tile([HID, HID], bf16)
            nc.vector.tensor_copy(wr2b_t[:], wr2_t[:])
            wout_t = cpool.tile([HID, HID], f32); nc.sync.dma_start(wout_t[:], wout_p[:])
            bl1_t = cpool.tile([HID, 1], f32); nc.sync.dma_start(bl1_t[:], bl1_p[:])
            bl2_t = cpool.tile([HID, 1], f32); nc.sync.dma_start(bl2_t[:], bl2_p[:])
            bout_t = cpool.tile([128, CH * HID], f32); nc.sync.dma_start(bout_t[:], bout_p[:])

            h1T = bpool.tile([HID, NB, 128], bf16)     # h1 transposed, own nodes
            zt = cpool.tile([128, HID], f32)
            nc.vector.memset(zt[:], 0.0)
            zbig = cpool.tile([128, 11 * HID], f32)
            nc.vector.memset(zbig[:], 0.0)

            def zero_accs(li):
                for w in range(NW):
                    # ACCR = 12672 = 128 * 99; partition p covers rows
                    # [99p, 99p+99) contiguously
                    dstv = acc_d[li][w][:].rearrange("(p b) f -> p (b f)", p=128)
                    for c in range(9):
                        nc.sync.dma_start(
                            dstv[:, c * 11 * HID:(c + 1) * 11 * HID], zbig[:])


            def phase1_setup(gi_p, si_p):
                gi_t = bpool.tile([128, gi_p.shape[1]], i16, tag="gi", name="gi_t")
                si_t = bpool.tile([128, si_p.shape[1]], i16, tag="si", name="si_t")
                nc.sync.dma_start(gi_t[:], gi_p[:])
                nc.sync.dma_start(si_t[:], si_p[:])
                return {"gi": gi_t, "si": si_t, "gcol": 0, "scol": 0,
                        "pend": []}

            def p1_flush(st, li, n=None):
                si_t = st["si"]
                pend = st["pend"]
                k = len(pend) if n is None else n
                for _ in range(k):
                    r_p, nt_p, sc_p, w_p = pend.pop(0)
                    nc.gpsimd.dma_scatter_add(
                        acc_d[li][w_p][:], r_p[:, :nt_p, :],
                        si_t[:, sc_p:sc_p + nt_p * 8],
                        nt_p * 128, nt_p * 128, HID, single_packet=False,
                        queue_num=next_q(nt_p * 128))

            def phase1_window(st, li, groups, D, w, win, depth=2):
                gi_t, si_t = st["gi"], st["si"]
                gcol, scol = st["gcol"], st["scol"]
                pend = st["pend"]

                for gt in groups[w]:
                    npos = int(sum(D[w, t] for t in gt)) * 128
                    ncols = npos // 128
                    nt = len(gt)
                    g_t = gpool.tile([128, MAXPOS // 128, HID], f32, tag="g", name="g_t")
                    r_t = gpool.tile([128, MAXT, HID], f32, tag="r", name="r_t")
                    nc.gpsimd.dma_gather(
                        g_t[:, :ncols, :], win, gi_t[:, gcol:gcol + npos // 16],
                        npos, npos, HID, single_packet=False,
                        queue_num=next_q(npos))
                    off = 0
                    for i, t in enumerate(gt):
                        d = int(D[w, t])
                        view = g_t[:, off:off + d, :].rearrange("p d f -> p f d")
                        nc.vector.tensor_reduce(r_t[:, i, :], view, axis=AX, op=ADD)
                        off += d
                    pend.append((r_t, nt, scol, w))
                    gcol += npos // 16
                    scol += nt * 8
                    if len(pend) > depth:
                        p1_flush(st, li, 1)
                st["gcol"], st["scol"] = gcol, scol

            def phase2(L, chunks=None):
                wl_t = wl1_t if L == 1 else wl2_t
                wr_t = wr1_t if L == 1 else wr2_t
                bl_t = bl1_t if L == 1 else bl2_t
                alpha = a1f if L == 1 else a2f
                if chunks is None:
                    chunks = [(c, min(CH, NB - c)) for c in range(0, NB, CH)]
                for c0, ct in chunks:
                    m4 = qpool.tile([128, NW, CH, HID], f32, tag="m4", name="m4")
                    for w in range(NW):
                        (nc.sync if w % 2 else nc.scalar).dma_start(
                            m4[:, w, :ct, :],
                            acc_d[L - 1][w][c0 * 128:(c0 + ct) * 128]
                            .rearrange("(t p) f -> p t f", p=128))
                    invcc = qpool.tile([128, CH, HID], f32, tag="invcc", name="invcc")
                    nc.scalar.dma_start(invcc[:, :ct, :], invce_p[:, c0:c0 + ct, :])
                    mean0 = qpool.tile([128, CH, HID], f32, tag="mean0", name="mean0")
                    nc.vector.tensor_reduce(
                        mean0[:, :ct, :],
                        m4[:, :, :ct, :].rearrange("p w t f -> p t f w"),
                        axis=AX, op=ADD)
                    nc.vector.tensor_tensor(
                        mean0[:, :ct, :].rearrange("p t f -> p (t f)"),
                        mean0[:, :ct, :].rearrange("p t f -> p (t f)"),
                        invcc[:, :ct, :].rearrange("p t f -> p (t f)"),
                        op=mybir.AluOpType.mult)
                    # transpose ct tiles into psum (two banks of 4 tiles)
                    meanT = qpool.tile([HID, CH * 128], f32, tag="meanT", name="meanT")
                    for hb in range(0, ct, 4):
                        hn = min(4, ct - hb)
                        psT = ppool.tile([HID, 512], f32, tag=f"psT{(hb // 4) % 2}",
                                         name="psT")
                        for i in range(hn):
                            nc.tensor.transpose(
                                psT[:, i * 128:(i + 1) * 128],
                                mean0[:, hb + i, :], ident[:])
                        nc.vector.tensor_copy(
                            meanT[:, hb * 128:(hb + hn) * 128], psT[:, :hn * 128])
                    if L == 1:
                        hT = qpool.tile([HID, CH * 128], f32, tag="hT", name="hT")
                        nc.sync.dma_start(
                            hT[:, :ct * 128],
                            h0T_p[:, c0 * 128:(c0 + ct) * 128])
                    # matmuls in half-chunks of 4 tiles (psum 512 col limit)
                    for h in range(0, ct, 4):
                        hw = min(4, ct - h)
                        cols = slice(h * 128, (h + hw) * 128)
                        psC = ppool.tile([HID, 512], f32, tag=f"psC{(h // 4) % 2}",
                                         name="psC")
                        nc.tensor.matmul(psC[:, :hw * 128], wl_t[:], meanT[:, cols],
                                         start=True, stop=False)
                        if L == 1:
                            nc.tensor.matmul(psC[:, :hw * 128], wr_t[:], hT[:, cols],
                                             start=False, stop=True)
                        else:
                            nc.tensor.matmul(
                                psC[:, :hw * 128], wr2b_t[:],
                                h1T[:, c0 + h:c0 + h + hw, :]
                                .rearrange("f t n -> f (t n)"),
                                start=False, stop=True)
                        act_out = (h1T[:, c0 + h:c0 + h + hw, :] if L == 1 else
                                   h2T_t[:, h:h + hw, :])
                        nc.scalar.activation(
                            act_out.rearrange("f t n -> f (t n)"),
                            psC[:, :hw * 128], PRELU, bias=bl_t[:], alpha=alpha)
                    if L == 1:
                        # back-transpose to node-major and write hc1
                        psH = ppool.tile([128, CH * HID], bf16, tag="psH", name="psH")
                        for i in range(ct):
                            nc.tensor.transpose(
                                psH[:, i * HID:(i + 1) * HID],
                                h1T[:, c0 + i, :], ident_bf[:])
                        hc1c = qpool.tile([128, CH, HID], f32, tag="hc1c", name="hc1c")
                        nc.vector.tensor_copy(
                            hc1c[:, :ct, :].rearrange("p t f -> p (t f)"),
                            psH[:, :ct * HID])
                        # write into quarter tensors (split at boundaries)
                        r0, r1 = c0 * 128, (c0 + ct) * 128
                        for q in range(NW):
                            q0, q1 = QS[q], QS[q] + QR[q]
                            a, b = max(r0, q0), min(r1, q1)
                            if a >= b:
                                continue
                            t_a = (a - r0) // 128
                            t_b = (b - r0) // 128
                            nc.sync.dma_start(
                                hc1q[q][a - q0:b - q0]
                                .rearrange("(t p) f -> p t f", p=128),
                                hc1c[:, t_a:t_b, :])
                        if c0 + ct in (25, 50, 75):
                            qq = (c0 + ct) // 25 - 1
                            nc.gpsimd.collective_compute(
                                "AllGather", mybir.AluOpType.bypass,
                                replica_groups=[list(range(P))],
                                ins=[hc1q[qq][:]],
                                outs=[hcatq[qq][:P * QR[qq]]])
                    else:
                        psE = ppool.tile([128, CH * HID], f32, tag="psE", name="psE")
                        for i in range(ct):
                            nc.tensor.matmul(
                                psE[:, i * HID:(i + 1) * HID],
                                h2T_t[:, i, :], wout_t[:], start=True, stop=True)
                        outc = qpool.tile([128, CH, HID], f32, tag="outc", name="outc")
                        nc.vector.tensor_tensor(
                            outc[:, :ct, :].rearrange("p t f -> p (t f)"),
                            psE[:, :ct * HID], bout_t[:, :ct * HID], op=ADD)
                        nc.sync.dma_start(
                            out_p[c0 * 128:(c0 + ct) * 128]
                            .rearrange("(t p) f -> p t f", p=128),
                            outc[:, :ct, :])

            h2T_t = bpool.tile([HID, CH, 128], f32)

            zero_accs(0)
            # zero the per-window pad blocks of the shared gather sources
            for q in range(NW):
                nc.sync.dma_start(hcatq[q][P * QR[q]:], zt[:])
            # ---- layer 1 ----
            st1 = phase1_setup(gi1, si1)
            for w in range(NW):
                phase1_window(st1, 0, g1, D1, w, embw[w * WIN:(w + 1) * WIN])
            p1_flush(st1, 0)
            zero_accs(1)
            phase2(1, chunks=[(0, 8), (8, 8), (16, 8), (24, 1),
                              (25, 8), (33, 8), (41, 8), (49, 1),
                              (50, 8), (58, 8), (66, 8), (74, 1),
                              (75, 8), (83, 8), (91, 7)])
            # L2 windows 0-2 only need collectives 0-2 (fired inline above)
            st2 = phase1_setup(gi2, si2)
            for w in range(3):
                phase1_window(st2, 1, g2, D2, w, hcatq[w][:])
            # zero pad rows of h1 (nodes 12500..12543) so gather pads stay 0
            nc.sync.dma_start(hc1q[3][NPC - QS[3]:], zt[:RPC - NPC, :])
            nc.gpsimd.collective_compute(
                "AllGather", mybir.AluOpType.bypass,
                replica_groups=[list(range(P))],
                ins=[hc1q[3][:]],
                outs=[hcatq[3][:P * QR[3]]])
            phase1_window(st2, 1, g2, D2, 3, hcatq[3][:])
            p1_flush(st2, 1)
            phase2(2)

    nc.compile()

    in_maps = []
    for k in range(P):
        in_maps.append({
            "embw": emb_hc, "h0T": h0T_own[k],
            "gi1": gidx1[k], "si1": sidx1[k],
            "gi2": gidx2[k], "si2": sidx2[k],
            "invce": np.repeat(invc[k].reshape(128, NB, 1), HID, axis=2),
            "wl1": Wl1, "wr1": Wr1, "wl2": Wl2, "wr2": Wr2, "wout": Wout,
            "bl1t": bl1.reshape(HID, 1), "bl2t": bl2.reshape(HID, 1),
            "boutr": np.tile(bout.reshape(1, HID), (128, CH)),
        })
    res = run_bass_kernel_spmd(nc, in_maps, list(range(P)))
    out = np.zeros((N, HID), np.float32)
    for k in range(P):
        out[k * NPC:(k + 1) * NPC] = res.results[k]["out"][:NPC]
    kernel.last_exec_time_ns = res.exec_time_ns
    return out
